# revision 29
# baseline (speedup 1.0000x reference)
"""Masked attention (B=4, M=N=4096, D=64) on 8 Trainium2 NeuronCores.

Sharding: batch (4) x m-halves (2) -> 8 cores, no cross-core communication.
Each core computes out[m, :] = softmax(mask(q@k^T)/sqrt(d)) @ v for its
2048 q rows against the full 4096 k/v rows of its batch.

Device algorithm (per core). The PE streaming floor is 54.6us (QK 27.3 +
PV 27.3 at 512 moving cols / 213ns); fp8 matmuls are numerically dead for
this problem (any p-perturbation >=1.5% rms blows the 2e-2 gate), so the
win over the previous version comes from balancing the exp+mask stage over
THREE engines (ScalarE + VectorE + GpSimd) so the stage (~43us/engine)
stays off the PE critical path, and from 1-byte masks on the V/G routes
(mask DMA 14MB -> 11MB).

  - q/k host-scaled by sqrt(1024*log2e/8); QK produces S^T (n on
    partitions) in fp16-exponent-bit units (1477.32 * z) in PSUM f32.
  - Per 128-row n-chunk, one of three exp+mask routes (route table below,
    chosen pair-wise so each mask DMA stays one >=2KB-per-partition
    transfer):
    * V route (12 chunks/half): one custom VectorE op (EXP2_BITS_ANT)
      computes quadratic-mantissa-corrected 2^x fp16 BITS (u16) with the
      softmax bias AND mask folded into the additive fp8e5 operand mb8 in
      {12288 (masked => 2^-8 suppression), 20480 (keep)}. exp+mask in one
      1-byte-mask DVE pass; the u16 IS the fp16 attention weight.
    * SV route (10 chunks/half): ScalarE e = Exp(S/1477.32 + ln32) fp16,
      then VectorE p = e * nm16 (fp16 {0,1} mask, 2x DVE mode).
    * SG route (10 chunks/half): ScalarE exp as above, then GpSimd
      p = e * nm8 (fp8e4 {0,1} mask, 1 byte) -- uses the otherwise-idle
      Pool engine.
  - Matmuls stay at 512 moving columns (MAX_MOVING_FREE_DIM_SIZE is real).
  - PV: out^T[j, m] += v_aug_chunk.T @ p (v_aug = [v | ones]; row 64 of
    out^T = softmax denominator for free), software-pipelined one pair
    behind QK.
  - The out^T [65, 1024] accumulators ship raw; host divides/transposes.
  - Dense K=128 keepalive matmul bursts pin the PE HAM clock-gate at 8/8
    (2.4 GHz); K=64 QK matmuls do not trip the activity monitor.
"""

import numpy as np
import ml_dtypes
from contextlib import ExitStack

import concourse.bacc as bacc
import concourse.mybir as mybir
import concourse.tile as tile
from concourse.bass_utils import run_bass_kernel_spmd

B, M, N, D = 4, 4096, 4096, 64
NCORES = 8
M_LOC = M // 2        # q rows per core
MH = 1024             # m sub-block held in one PSUM accumulation
NCH = N // 128        # 32 n-chunks of 128
NPAIR = NCH // 2      # 16 chunk-pairs

# Pair-level route table (per half): V = custom-DVE fused exp+mask,
# S = ScalarE exp + VectorE mul (fp16 mask), G = ScalarE exp + GpSimd mul
# (fp8e4 mask). Pattern keeps per-engine load even through the schedule and
# ends on a V pair (shortest dependency chain -> short tail).
_PAIR_ROUTE = ['V', 'S', 'V', 'S', 'V', 'G', 'S', 'G', 'V', 'G', 'S', 'G',
               'V', 'G', 'S', 'V']
_CHUNK_ROUTE = []
for _r in _PAIR_ROUTE:
    _CHUNK_ROUTE += [_r, _r]
N_V = _CHUNK_ROUTE.count('V')
N_S = _CHUNK_ROUTE.count('S')
N_G = _CHUNK_ROUTE.count('G')

LOG2E = float(np.log2(np.e))
SQ = float(np.sqrt(1024.0 * LOG2E / 8.0))   # 13.5874: q/k pre-scale
MAGIC = 3.0 * 2.0**32                        # RNE-to-multiple-of-1024 magic
B_MASK, B_KEEP = 12288.0, 20480.0            # additive fp8e5 mask biases
GAMMA = -0.346                               # quadratic mantissa correction
ACT_SCALE = 1.0 / (1024.0 * LOG2E)
ACT_BIAS = (B_KEEP - 15360.0) / (1024.0 * LOG2E)   # +3.46574 = ln 32

BF16 = mybir.dt.bfloat16
F32 = mybir.dt.float32
FP16 = mybir.dt.float16
FP8E4 = mybir.dt.float8e4
FP8E5 = mybir.dt.float8e5
U16 = mybir.dt.uint16
E4 = ml_dtypes.float8_e4m3fn
E5 = ml_dtypes.float8_e5m2

_NC = None
_EXP2_OP = None
LAST_RESULTS = None   # BassKernelResults of the most recent run (for profiling)
TRACE = False
TRACE_KW = {}


def _get_exp2_op():
    """Register (once) the corrected-exp2 custom DVE op."""
    global _EXP2_OP
    if _EXP2_OP is not None:
        return _EXP2_OP
    from concourse.dve_spec import C0, C1, C2, AluOp, Bin, Spec, Src0, Src1, lower
    from concourse.dve_spec import _has_src1
    from concourse.dve_uop import DveOpSpec
    from concourse import dve_ops as dops

    t = Src0 + Src1
    r = (t + C0) - C0
    d = Bin(AluOp.ABSOLUTE_DIFF, t, r)
    body = t + (d * (C1 - d)) * C2

    def ref(in0, in1, s0, s1, imm2):
        tt = (in0.astype(np.float32) + in1.astype(np.float32)).astype(np.float32)
        rr = ((tt + np.float32(s0)).astype(np.float32) - np.float32(s0)).astype(
            np.float32)
        dd = np.abs((tt - rr).astype(np.float32))
        return (tt + dd * (np.float32(s1) - dd) * np.float32(imm2)).astype(
            np.float32)

    spec = Spec(body=body, reference=ref)
    name = "EXP2_BITS_ANT"
    if name not in dops._SUB_OPCODE_FOR_NAME:
        row = max(dops._SUB_OPCODE_FOR_NAME.values()) + 1
        assert row < 0x20
        dops._SUB_OPCODE_FOR_NAME[name] = row
        sha = DveOpSpec(name=name, opcode=row, uops=lower(spec, ver="v3"),
                        rd1_en=_has_src1(spec)).sha("v3")
        op = dops.DveOp(name=name, spec=spec, subdim=False,
                        uops_sha={"v3": sha})
        dops.OPS.append(op)
        dops.CUSTOM_DVE_SPECS[name] = spec
        _EXP2_OP = op
    else:
        _EXP2_OP = next(o for o in dops.OPS if o.name == name)
    return _EXP2_OP


def _build_nc():
    exp2_op = _get_exp2_op()
    nc = bacc.Bacc("TRN2", target_bir_lowering=False, debug=False,
                   num_devices=NCORES)
    # q zero-padded to 128 rows on host (rows 64-127 = 0) to match the
    # duplicated k rows: full-array MAC density, numerically exact.
    qT = nc.dram_tensor("qT", [128, M_LOC], FP16, kind="ExternalInput").ap()
    # k rows duplicated to 128 partitions; matched against zeroed q rows
    # 64-127 so the QK matmul presents full-array MAC density to the HAM
    # clock monitor (zero-padding is numerically exact, streaming cost
    # unchanged).
    kT = nc.dram_tensor("kT", [128, N], FP16, kind="ExternalInput").ap()
    # v_aug chunks padded from 65 to 128 stationary columns (junk v copies)
    # so PV also looks dense; PSUM rows 65-127 accumulate garbage never read.
    vA = nc.dram_tensor("vA", [128, NCH * 128], FP16,
                        kind="ExternalInput").ap()
    mbV = nc.dram_tensor("mbV", [N_V * 128, M_LOC], FP8E5,
                         kind="ExternalInput").ap()
    nmS = nc.dram_tensor("nmS", [N_S * 128, M_LOC], FP16,
                         kind="ExternalInput").ap()
    nmG = nc.dram_tensor("nmG", [N_G * 128, M_LOC], FP8E4,
                         kind="ExternalInput").ap()
    # raw accumulator output: out^T with the softmax denominator in row 64
    o = nc.dram_tensor("oT", [2, D + 1, MH], F32, kind="ExternalOutput").ap()

    with tile.TileContext(nc) as tc, ExitStack() as ctx:
        const = ctx.enter_context(tc.tile_pool(name="const", bufs=1))
        vpool = ctx.enter_context(tc.tile_pool(name="maskv", bufs=3))
        spool_m = ctx.enter_context(tc.tile_pool(name="masks", bufs=3))
        gpool_m = ctx.enter_context(tc.tile_pool(name="maskg", bufs=3))
        epool = ctx.enter_context(tc.tile_pool(name="e", bufs=8))
        ppool = ctx.enter_context(tc.tile_pool(name="p", bufs=10))
        fpool = ctx.enter_context(tc.tile_pool(name="fin", bufs=2))
        spool = ctx.enter_context(tc.tile_pool(name="spsum", bufs=3, space="PSUM"))
        opool = ctx.enter_context(tc.tile_pool(name="opsum", bufs=1, space="PSUM"))

        # spread the constant loads over DMA queues so they overlap; the
        # first QK pair only needs qT + kT[:, 0:256], so kT's head gets its
        # own small transfer.
        qT_s = const.tile([128, M_LOC], FP16)
        nc.sync.dma_start(qT_s[:], qT)
        kT_s = const.tile([128, N], FP16)
        nc.scalar.dma_start(kT_s[:, 0:512], kT[:, 0:512])
        nc.gpsimd.dma_start(kT_s[:, 512:2048], kT[:, 512:2048])
        nc.gpsimd.dma_start(kT_s[:, 2048:N], kT[:, 2048:N])
        vA_s = const.tile([128, NCH * 128], FP16)
        nc.scalar.dma_start(vA_s[:], vA)
        ebias = const.tile([128, 1], F32)
        nc.vector.memset(ebias[:], ACT_BIAS)
        # warmup operand with no DMA dependency (starts right after preamble)
        wsrc = const.tile([128, 512], BF16)
        nc.gpsimd.memset(wsrc[:], 1.0)

        # Dense back-to-back full-array (K=128) matmuls keep the PE HAM
        # clock-gate at 8/8 (results discarded into a rotating S slot).
        def pe_keepalive(n):
            wu = spool.tile([128, MH], F32, tag="s")
            for i in range(n):
                nc.tensor.matmul(wu[:, 0:512], wsrc[:, 0:128], wsrc[:, 0:512],
                                 start=True, stop=True)

        for h in range(2):
            if h == 0:
                pe_keepalive(6)
            o_ps = opool.tile([128, MH], F32)
            # PV runs PV_LAG pairs behind QK so stage bursts (a G pair is
            # ~4us of Pool work) never stall the PE.
            PV_LAG = 4
            pv_queue = []        # list of per-pair lists

            def flush_pv(drain=False):
                while pv_queue and (drain or len(pv_queue) > PV_LAG):
                    for ni, pap in pv_queue.pop(0):
                        vch = vA_s[:, ni * 128:(ni + 1) * 128]
                        nc.tensor.matmul(o_ps[:, 0:512], vch, pap[:, 0:512],
                                         start=(ni == 0), stop=(ni == NCH - 1))
                        nc.tensor.matmul(o_ps[:, 512:1024], vch,
                                         pap[:, 512:1024],
                                         start=(ni == 0), stop=(ni == NCH - 1))

            pv_pending = []

            def route_v(ni, S, mb_sl):
                pb = ppool.tile([128, MH], U16)
                nc.vector._custom_dve(exp2_op, out=pb[:], in0=S[:], in1=mb_sl,
                                      s0=MAGIC, s1=1024.0, imm2=GAMMA / 1024.0)
                pv_pending.append((ni, pb[:].bitcast(FP16)))

            def route_sv(ni, S, nm_sl):
                e = epool.tile([128, MH], FP16)
                nc.scalar.activation(e[:], S[:],
                                     mybir.ActivationFunctionType.Exp,
                                     bias=ebias[:], scale=ACT_SCALE)
                p = ppool.tile([128, MH], FP16)
                nc.vector.tensor_mul(p[:], e[:], nm_sl)
                pv_pending.append((ni, p[:]))

            def route_sg(ni, S, nm_sl):
                e = epool.tile([128, MH], FP16)
                nc.scalar.activation(e[:], S[:],
                                     mybir.ActivationFunctionType.Exp,
                                     bias=ebias[:], scale=ACT_SCALE)
                p = ppool.tile([128, MH], FP16)
                nc.gpsimd.tensor_mul(p[:], e[:], nm_sl)
                pv_pending.append((ni, p[:]))

            # mask DMA for pair pc, prefetched PREF pairs ahead of use
            PREF = 2
            vj_c, sj_c, gj_c = [0], [0], [0]
            mask_tiles = {}

            def issue_mask(pc):
                if pc >= NPAIR:
                    return
                r = _PAIR_ROUTE[pc]
                if r == 'V':
                    t = vpool.tile([128, 2 * MH], FP8E5)
                    src_t, cnt = mbV, vj_c
                elif r == 'S':
                    t = spool_m.tile([128, 2 * MH], FP16)
                    src_t, cnt = nmS, sj_c
                else:
                    t = gpool_m.tile([128, 2 * MH], FP8E4)
                    src_t, cnt = nmG, gj_c
                j = cnt[0]
                src = src_t[j * 128:(j + 2) * 128,
                            h * MH:(h + 1) * MH].rearrange(
                                "(t p) m -> p t m", t=2)
                nc.sync.dma_start(t[:].rearrange("p (t m) -> p t m", t=2), src)
                cnt[0] += 2
                mask_tiles[pc] = t

            for pc in range(PREF):
                issue_mask(pc)

            for pc in range(NPAIR):
                c0, c1 = 2 * pc, 2 * pc + 1
                rhs = qT_s[:, h * MH:(h + 1) * MH]
                lhs0 = kT_s[:, c0 * 128:(c0 + 1) * 128]
                lhs1 = kT_s[:, c1 * 128:(c1 + 1) * 128]
                S0 = spool.tile([128, MH], F32, tag="s")
                S1 = spool.tile([128, MH], F32, tag="s")
                nc.tensor.matmul(S0[:, 0:512], lhs0, rhs[:, 0:512],
                                 start=True, stop=True)
                nc.tensor.matmul(S0[:, 512:1024], lhs0, rhs[:, 512:1024],
                                 start=True, stop=True)
                nc.tensor.matmul(S1[:, 0:512], lhs1, rhs[:, 0:512],
                                 start=True, stop=True)
                nc.tensor.matmul(S1[:, 512:1024], lhs1, rhs[:, 512:1024],
                                 start=True, stop=True)
                # PV of the pair PV_LAG back goes right after this pair's QK
                # so the PE never waits on recent exp/mask chains.
                if pv_pending:
                    pv_queue.append(list(pv_pending))
                    pv_pending.clear()
                flush_pv()
                issue_mask(pc + PREF)
                r = _PAIR_ROUTE[pc]
                mt = mask_tiles.pop(pc)
                if r == 'V':
                    route_v(c0, S0, mt[:, 0:MH])
                    route_v(c1, S1, mt[:, MH:2 * MH])
                elif r == 'S':
                    route_sv(c0, S0, mt[:, 0:MH])
                    route_sv(c1, S1, mt[:, MH:2 * MH])
                else:
                    route_sg(c0, S0, mt[:, 0:MH])
                    route_sg(c1, S1, mt[:, MH:2 * MH])
            if pv_pending:
                pv_queue.append(list(pv_pending))
                pv_pending.clear()
            flush_pv(drain=True)
            oT = fpool.tile([D + 1, MH], F32)
            nc.scalar.copy(oT[:, 0:MH // 2], o_ps[0:D + 1, 0:MH // 2])
            nc.sync.dma_start(o[h, :, 0:MH // 2], oT[:, 0:MH // 2])
            nc.vector.tensor_copy(oT[:, MH // 2:MH], o_ps[0:D + 1, MH // 2:MH])
            nc.sync.dma_start(o[h, :, MH // 2:MH], oT[:, MH // 2:MH])
    nc.compile()
    return nc


def _get_nc():
    global _NC
    if _NC is None:
        _NC = _build_nc()
    return _NC


def _prep_core(q, k, v, mask, b, j):
    qs = q[b, j * M_LOC:(j + 1) * M_LOC, :]
    qT = np.zeros((128, M_LOC), np.float16)
    qT[0:64] = (qs.T * SQ).astype(np.float16)                  # rows 64-127 = 0
    kT1 = np.ascontiguousarray(k[b].T * SQ).astype(np.float16)  # [64, 4096]
    kT = np.concatenate([kT1, kT1], axis=0)                    # [128, 4096]
    vb = v[b]                                                  # [4096, 64]
    vA = np.empty((128, NCH * 128), np.float16)
    vAr = vA.reshape(128, NCH, 128)
    vch = vb.reshape(NCH, 128, D).transpose(1, 0, 2).astype(np.float16)
    vAr[:, :, :D] = vch
    vAr[:, :, D] = np.float16(1.0)
    vAr[:, :, D + 1:] = vch[:, :, :128 - D - 1]   # junk pad for PE density
    nm = ~mask[b, j * M_LOC:(j + 1) * M_LOC, :]                # [2048, 4096]
    nmT = np.ascontiguousarray(nm.T)                           # [4096, 2048]
    v_rows = np.concatenate(
        [nmT[c * 128:(c + 1) * 128] for c in range(NCH)
         if _CHUNK_ROUTE[c] == 'V'], axis=0)
    mbV = np.where(v_rows, np.float32(B_KEEP), np.float32(B_MASK)).astype(E5)
    s_rows = np.concatenate(
        [nmT[c * 128:(c + 1) * 128] for c in range(NCH)
         if _CHUNK_ROUTE[c] == 'S'], axis=0)
    nmS = s_rows.astype(np.float16)
    g_rows = np.concatenate(
        [nmT[c * 128:(c + 1) * 128] for c in range(NCH)
         if _CHUNK_ROUTE[c] == 'G'], axis=0)
    nmG = g_rows.astype(E4)
    return {"qT": qT, "kT": kT, "vA": vA, "mbV": mbV, "nmS": nmS, "nmG": nmG}


def kernel(q, k, v, mask):
    global LAST_RESULTS
    q = np.asarray(q, dtype=np.float32)
    k = np.asarray(k, dtype=np.float32)
    v = np.asarray(v, dtype=np.float32)
    mask = np.asarray(mask)
    nc = _get_nc()
    in_maps = [_prep_core(q, k, v, mask, c // 2, c % 2) for c in range(NCORES)]
    res = run_bass_kernel_spmd(nc, in_maps, core_ids=list(range(NCORES)),
                               trace=TRACE, **TRACE_KW)
    LAST_RESULTS = res
    out = np.empty((B, M, D), np.float32)
    for c in range(NCORES):
        b, j = divmod(c, 2)
        oT = res.results[c]["oT"]                      # [2, 65, MH]
        for h in range(2):
            blk = oT[h, :D, :] / oT[h, D, :]           # [64, MH]
            lo = j * M_LOC + h * MH
            out[b, lo:lo + MH, :] = blk.T
    return out


# revision 31
# speedup vs baseline: 1.1895x; 1.1895x over previous
"""Masked attention (B=4, M=N=4096, D=64) on 8 Trainium2 NeuronCores.

Sharding: batch (4) x m-halves (2) -> 8 cores, no cross-core communication.
Each core computes out[m, :] = softmax(mask(q@k^T)/sqrt(d)) @ v for its
2048 q rows against the full 4096 k/v rows of its batch.

Device algorithm (per core). The PE streaming floor is 54.6us (QK 27.3 +
PV 27.3 at 512 moving cols / 213ns); fp8 matmuls are numerically dead for
this problem (any p-perturbation >=1.5% rms blows the 2e-2 gate), so the
win over the previous version comes from balancing the exp+mask stage over
THREE engines (ScalarE + VectorE + GpSimd) so the stage (~43us/engine)
stays off the PE critical path, and from 1-byte masks on the V/G routes
(mask DMA 14MB -> 11MB).

  - q/k host-scaled by sqrt(1024*log2e/8); QK produces S^T (n on
    partitions) in fp16-exponent-bit units (1477.32 * z) in PSUM f32.
  - Per 128-row n-chunk, one of three exp+mask routes (route table below,
    chosen pair-wise so each mask DMA stays one >=2KB-per-partition
    transfer):
    * V route (12 chunks/half): one custom VectorE op (EXP2_BITS_ANT)
      computes quadratic-mantissa-corrected 2^x fp16 BITS (u16) with the
      softmax bias AND mask folded into the additive fp8e5 operand mb8 in
      {12288 (masked => 2^-8 suppression), 20480 (keep)}. exp+mask in one
      1-byte-mask DVE pass; the u16 IS the fp16 attention weight.
    * SV route (10 chunks/half): ScalarE e = Exp(S/1477.32 + ln32) fp16,
      then VectorE p = e * nm16 (fp16 {0,1} mask, 2x DVE mode).
    * SG route (10 chunks/half): ScalarE exp as above, then GpSimd
      p = e * nm8 (fp8e4 {0,1} mask, 1 byte) -- uses the otherwise-idle
      Pool engine.
  - Matmuls stay at 512 moving columns (MAX_MOVING_FREE_DIM_SIZE is real).
  - PV: out^T[j, m] += v_aug_chunk.T @ p (v_aug = [v | ones]; row 64 of
    out^T = softmax denominator for free), software-pipelined one pair
    behind QK.
  - The out^T [65, 1024] accumulators ship raw; host divides/transposes.
  - Dense K=128 keepalive matmul bursts pin the PE HAM clock-gate at 8/8
    (2.4 GHz); K=64 QK matmuls do not trip the activity monitor.
"""

import numpy as np
import ml_dtypes
from contextlib import ExitStack

import concourse.bacc as bacc
import concourse.mybir as mybir
import concourse.tile as tile
from concourse.bass_utils import run_bass_kernel_spmd

B, M, N, D = 4, 4096, 4096, 64
NCORES = 8
M_LOC = M // 2        # q rows per core
MH = 1024             # m sub-block held in one PSUM accumulation
NCH = N // 128        # 32 n-chunks of 128
NPAIR = NCH // 2      # 16 chunk-pairs

# Pair-level route table (per half): V = custom-DVE fused exp+mask,
# S = ScalarE exp + VectorE mul (fp16 mask), G = ScalarE exp + GpSimd mul
# (fp8e4 mask). Pattern keeps per-engine load even through the schedule and
# ends on a V pair (shortest dependency chain -> short tail).
_PAIR_ROUTE = ['V', 'S', 'V', 'G', 'S', 'G', 'V', 'S', 'G', 'V', 'S', 'G',
               'V', 'G', 'S', 'V']
_CHUNK_ROUTE = []
for _r in _PAIR_ROUTE:
    _CHUNK_ROUTE += [_r, _r]
N_V = _CHUNK_ROUTE.count('V')
N_S = _CHUNK_ROUTE.count('S')
N_G = _CHUNK_ROUTE.count('G')

LOG2E = float(np.log2(np.e))
SQ = float(np.sqrt(1024.0 * LOG2E / 8.0))   # 13.5874: q/k pre-scale
MAGIC = 3.0 * 2.0**32                        # RNE-to-multiple-of-1024 magic
B_MASK, B_KEEP = 12288.0, 20480.0            # additive fp8e5 mask biases
GAMMA = -0.346                               # quadratic mantissa correction
ACT_SCALE = 1.0 / (1024.0 * LOG2E)
ACT_BIAS = (B_KEEP - 15360.0) / (1024.0 * LOG2E)   # +3.46574 = ln 32

BF16 = mybir.dt.bfloat16
F32 = mybir.dt.float32
FP16 = mybir.dt.float16
FP8E4 = mybir.dt.float8e4
FP8E5 = mybir.dt.float8e5
U16 = mybir.dt.uint16
E4 = ml_dtypes.float8_e4m3fn
E5 = ml_dtypes.float8_e5m2

_NC = None
_EXP2_OP = None
LAST_RESULTS = None   # BassKernelResults of the most recent run (for profiling)
TRACE = False
TRACE_KW = {}


def _get_exp2_op():
    """Register (once) the corrected-exp2 custom DVE op."""
    global _EXP2_OP
    if _EXP2_OP is not None:
        return _EXP2_OP
    from concourse.dve_spec import C0, C1, C2, AluOp, Bin, Spec, Src0, Src1, lower
    from concourse.dve_spec import _has_src1
    from concourse.dve_uop import DveOpSpec
    from concourse import dve_ops as dops

    t = Src0 + Src1
    r = (t + C0) - C0
    d = Bin(AluOp.ABSOLUTE_DIFF, t, r)
    body = t + (d * (C1 - d)) * C2

    def ref(in0, in1, s0, s1, imm2):
        tt = (in0.astype(np.float32) + in1.astype(np.float32)).astype(np.float32)
        rr = ((tt + np.float32(s0)).astype(np.float32) - np.float32(s0)).astype(
            np.float32)
        dd = np.abs((tt - rr).astype(np.float32))
        return (tt + dd * (np.float32(s1) - dd) * np.float32(imm2)).astype(
            np.float32)

    spec = Spec(body=body, reference=ref)
    name = "EXP2_BITS_ANT"
    if name not in dops._SUB_OPCODE_FOR_NAME:
        row = max(dops._SUB_OPCODE_FOR_NAME.values()) + 1
        assert row < 0x20
        dops._SUB_OPCODE_FOR_NAME[name] = row
        sha = DveOpSpec(name=name, opcode=row, uops=lower(spec, ver="v3"),
                        rd1_en=_has_src1(spec)).sha("v3")
        op = dops.DveOp(name=name, spec=spec, subdim=False,
                        uops_sha={"v3": sha})
        dops.OPS.append(op)
        dops.CUSTOM_DVE_SPECS[name] = spec
        _EXP2_OP = op
    else:
        _EXP2_OP = next(o for o in dops.OPS if o.name == name)
    return _EXP2_OP


def _build_nc():
    exp2_op = _get_exp2_op()
    nc = bacc.Bacc("TRN2", target_bir_lowering=False, debug=False,
                   num_devices=NCORES)
    # q zero-padded to 128 rows on host (rows 64-127 = 0) to match the
    # duplicated k rows: full-array MAC density, numerically exact.
    qT = nc.dram_tensor("qT", [128, M_LOC], FP16, kind="ExternalInput").ap()
    # k rows duplicated to 128 partitions; matched against zeroed q rows
    # 64-127 so the QK matmul presents full-array MAC density to the HAM
    # clock monitor (zero-padding is numerically exact, streaming cost
    # unchanged).
    kT = nc.dram_tensor("kT", [128, N], FP16, kind="ExternalInput").ap()
    # v_aug chunks padded from 65 to 128 stationary columns (junk v copies)
    # so PV also looks dense; PSUM rows 65-127 accumulate garbage never read.
    vA = nc.dram_tensor("vA", [128, NCH * 128], FP16,
                        kind="ExternalInput").ap()
    mbV = nc.dram_tensor("mbV", [N_V * 128, M_LOC], FP8E5,
                         kind="ExternalInput").ap()
    nmS = nc.dram_tensor("nmS", [N_S * 128, M_LOC], FP16,
                         kind="ExternalInput").ap()
    nmG = nc.dram_tensor("nmG", [N_G * 128, M_LOC], FP8E4,
                         kind="ExternalInput").ap()
    # raw accumulator output: out^T with the softmax denominator in row 64
    o = nc.dram_tensor("oT", [2, D + 1, MH], F32, kind="ExternalOutput").ap()

    with tile.TileContext(nc) as tc, ExitStack() as ctx:
        const = ctx.enter_context(tc.tile_pool(name="const", bufs=1))
        vpool = ctx.enter_context(tc.tile_pool(name="maskv", bufs=3))
        spool_m = ctx.enter_context(tc.tile_pool(name="masks", bufs=3))
        gpool_m = ctx.enter_context(tc.tile_pool(name="maskg", bufs=3))
        epool = ctx.enter_context(tc.tile_pool(name="e", bufs=8))
        ppool = ctx.enter_context(tc.tile_pool(name="p", bufs=8))
        fpool = ctx.enter_context(tc.tile_pool(name="fin", bufs=2))
        spool = ctx.enter_context(tc.tile_pool(name="spsum", bufs=3, space="PSUM"))
        opool = ctx.enter_context(tc.tile_pool(name="opsum", bufs=1, space="PSUM"))

        # spread the constant loads over DMA queues so they overlap; the
        # first QK pair only needs qT + kT[:, 0:256], so kT's head gets its
        # own small transfer.
        qT_s = const.tile([128, M_LOC], FP16)
        nc.sync.dma_start(qT_s[:], qT)
        kT_s = const.tile([128, N], FP16)
        nc.scalar.dma_start(kT_s[:, 0:512], kT[:, 0:512])
        nc.gpsimd.dma_start(kT_s[:, 512:2048], kT[:, 512:2048])
        nc.gpsimd.dma_start(kT_s[:, 2048:N], kT[:, 2048:N])
        vA_s = const.tile([128, NCH * 128], FP16)
        nc.scalar.dma_start(vA_s[:], vA)
        ebias = const.tile([128, 1], F32)
        nc.vector.memset(ebias[:], ACT_BIAS)
        # warmup operand with no DMA dependency (starts right after preamble)
        wsrc = const.tile([128, 512], BF16)
        nc.gpsimd.memset(wsrc[:], 1.0)

        # Dense back-to-back full-array (K=128) matmuls keep the PE HAM
        # clock-gate at 8/8 (results discarded into a rotating S slot).
        def pe_keepalive(n):
            wu = spool.tile([128, MH], F32, tag="s")
            for i in range(n):
                nc.tensor.matmul(wu[:, 0:512], wsrc[:, 0:128], wsrc[:, 0:512],
                                 start=True, stop=True)

        for h in range(2):
            if h == 0:
                pe_keepalive(10)
            o_ps = opool.tile([128, MH], F32)
            # PV runs PV_LAG pairs behind QK so stage bursts (a G pair is
            # ~4us of Pool work) never stall the PE.
            PV_LAG = 3
            pv_queue = []        # list of per-pair lists

            def flush_pv(drain=False):
                while pv_queue and (drain or len(pv_queue) > PV_LAG):
                    for ni, pap in pv_queue.pop(0):
                        vch = vA_s[:, ni * 128:(ni + 1) * 128]
                        nc.tensor.matmul(o_ps[:, 0:512], vch, pap[:, 0:512],
                                         start=(ni == 0), stop=(ni == NCH - 1))
                        nc.tensor.matmul(o_ps[:, 512:1024], vch,
                                         pap[:, 512:1024],
                                         start=(ni == 0), stop=(ni == NCH - 1))

            pv_pending = []

            def route_v(ni, S, mb_sl):
                pb = ppool.tile([128, MH], U16)
                nc.vector._custom_dve(exp2_op, out=pb[:], in0=S[:], in1=mb_sl,
                                      s0=MAGIC, s1=1024.0, imm2=GAMMA / 1024.0)
                pv_pending.append((ni, pb[:].bitcast(FP16)))

            def route_sv(ni, S, nm_sl):
                e = epool.tile([128, MH], FP16)
                nc.scalar.activation(e[:], S[:],
                                     mybir.ActivationFunctionType.Exp,
                                     bias=ebias[:], scale=ACT_SCALE)
                p = ppool.tile([128, MH], FP16)
                nc.vector.tensor_mul(p[:], e[:], nm_sl)
                pv_pending.append((ni, p[:]))

            def route_sg(ni, S, nm_sl):
                e = epool.tile([128, MH], FP16)
                nc.scalar.activation(e[:], S[:],
                                     mybir.ActivationFunctionType.Exp,
                                     bias=ebias[:], scale=ACT_SCALE)
                p = ppool.tile([128, MH], FP16)
                nc.gpsimd.tensor_mul(p[:], e[:], nm_sl)
                pv_pending.append((ni, p[:]))

            # mask DMA for pair pc, prefetched PREF pairs ahead of use
            PREF = 2
            vj_c, sj_c, gj_c = [0], [0], [0]
            mask_tiles = {}

            def issue_mask(pc):
                if pc >= NPAIR:
                    return
                r = _PAIR_ROUTE[pc]
                if r == 'V':
                    t = vpool.tile([128, 2 * MH], FP8E5)
                    src_t, cnt = mbV, vj_c
                elif r == 'S':
                    t = spool_m.tile([128, 2 * MH], FP16)
                    src_t, cnt = nmS, sj_c
                else:
                    t = gpool_m.tile([128, 2 * MH], FP8E4)
                    src_t, cnt = nmG, gj_c
                j = cnt[0]
                src = src_t[j * 128:(j + 2) * 128,
                            h * MH:(h + 1) * MH].rearrange(
                                "(t p) m -> p t m", t=2)
                nc.sync.dma_start(t[:].rearrange("p (t m) -> p t m", t=2), src)
                cnt[0] += 2
                mask_tiles[pc] = t

            for pc in range(PREF):
                issue_mask(pc)

            for pc in range(NPAIR):
                c0, c1 = 2 * pc, 2 * pc + 1
                rhs = qT_s[:, h * MH:(h + 1) * MH]
                lhs0 = kT_s[:, c0 * 128:(c0 + 1) * 128]
                lhs1 = kT_s[:, c1 * 128:(c1 + 1) * 128]
                S0 = spool.tile([128, MH], F32, tag="s")
                S1 = spool.tile([128, MH], F32, tag="s")
                nc.tensor.matmul(S0[:, 0:512], lhs0, rhs[:, 0:512],
                                 start=True, stop=True)
                nc.tensor.matmul(S0[:, 512:1024], lhs0, rhs[:, 512:1024],
                                 start=True, stop=True)
                nc.tensor.matmul(S1[:, 0:512], lhs1, rhs[:, 0:512],
                                 start=True, stop=True)
                nc.tensor.matmul(S1[:, 512:1024], lhs1, rhs[:, 512:1024],
                                 start=True, stop=True)
                # PV of the pair PV_LAG back goes right after this pair's QK
                # so the PE never waits on recent exp/mask chains.
                if pv_pending:
                    pv_queue.append(list(pv_pending))
                    pv_pending.clear()
                flush_pv()
                issue_mask(pc + PREF)
                r = _PAIR_ROUTE[pc]
                mt = mask_tiles.pop(pc)
                if r == 'V':
                    route_v(c0, S0, mt[:, 0:MH])
                    route_v(c1, S1, mt[:, MH:2 * MH])
                elif r == 'S':
                    route_sv(c0, S0, mt[:, 0:MH])
                    route_sv(c1, S1, mt[:, MH:2 * MH])
                else:
                    route_sg(c0, S0, mt[:, 0:MH])
                    route_sg(c1, S1, mt[:, MH:2 * MH])
            if pv_pending:
                pv_queue.append(list(pv_pending))
                pv_pending.clear()
            flush_pv(drain=True)
            oT = fpool.tile([D + 1, MH], F32)
            nc.scalar.copy(oT[:, 0:MH // 2], o_ps[0:D + 1, 0:MH // 2])
            nc.sync.dma_start(o[h, :, 0:MH // 2], oT[:, 0:MH // 2])
            nc.vector.tensor_copy(oT[:, MH // 2:MH], o_ps[0:D + 1, MH // 2:MH])
            nc.sync.dma_start(o[h, :, MH // 2:MH], oT[:, MH // 2:MH])
    nc.compile()
    return nc


def _get_nc():
    global _NC
    if _NC is None:
        _NC = _build_nc()
    return _NC


def _prep_core(q, k, v, mask, b, j):
    qs = q[b, j * M_LOC:(j + 1) * M_LOC, :]
    qT = np.zeros((128, M_LOC), np.float16)
    qT[0:64] = (qs.T * SQ).astype(np.float16)                  # rows 64-127 = 0
    kT1 = np.ascontiguousarray(k[b].T * SQ).astype(np.float16)  # [64, 4096]
    kT = np.concatenate([kT1, kT1], axis=0)                    # [128, 4096]
    vb = v[b]                                                  # [4096, 64]
    vA = np.empty((128, NCH * 128), np.float16)
    vAr = vA.reshape(128, NCH, 128)
    vch = vb.reshape(NCH, 128, D).transpose(1, 0, 2).astype(np.float16)
    vAr[:, :, :D] = vch
    vAr[:, :, D] = np.float16(1.0)
    vAr[:, :, D + 1:] = vch[:, :, :128 - D - 1]   # junk pad for PE density
    nm = ~mask[b, j * M_LOC:(j + 1) * M_LOC, :]                # [2048, 4096]
    nmT = np.ascontiguousarray(nm.T)                           # [4096, 2048]
    v_rows = np.concatenate(
        [nmT[c * 128:(c + 1) * 128] for c in range(NCH)
         if _CHUNK_ROUTE[c] == 'V'], axis=0)
    mbV = np.where(v_rows, np.float32(B_KEEP), np.float32(B_MASK)).astype(E5)
    s_rows = np.concatenate(
        [nmT[c * 128:(c + 1) * 128] for c in range(NCH)
         if _CHUNK_ROUTE[c] == 'S'], axis=0)
    nmS = s_rows.astype(np.float16)
    g_rows = np.concatenate(
        [nmT[c * 128:(c + 1) * 128] for c in range(NCH)
         if _CHUNK_ROUTE[c] == 'G'], axis=0)
    nmG = g_rows.astype(E4)
    return {"qT": qT, "kT": kT, "vA": vA, "mbV": mbV, "nmS": nmS, "nmG": nmG}


def kernel(q, k, v, mask):
    global LAST_RESULTS
    q = np.asarray(q, dtype=np.float32)
    k = np.asarray(k, dtype=np.float32)
    v = np.asarray(v, dtype=np.float32)
    mask = np.asarray(mask)
    nc = _get_nc()
    in_maps = [_prep_core(q, k, v, mask, c // 2, c % 2) for c in range(NCORES)]
    res = run_bass_kernel_spmd(nc, in_maps, core_ids=list(range(NCORES)),
                               trace=TRACE, **TRACE_KW)
    LAST_RESULTS = res
    out = np.empty((B, M, D), np.float32)
    for c in range(NCORES):
        b, j = divmod(c, 2)
        oT = res.results[c]["oT"]                      # [2, 65, MH]
        for h in range(2):
            blk = oT[h, :D, :] / oT[h, D, :]           # [64, MH]
            lo = j * M_LOC + h * MH
            out[b, lo:lo + MH, :] = blk.T
    return out
